# revision 21
# baseline (speedup 1.0000x reference)
"""InfoNCE loss kernel for Trainium2, 8 NeuronCores (v3).

Reference computation:
    z = l2_normalize(concat([polyline_embs, c_embs]))   # [8192, 512]
    sim = z @ z.T                                       # [8192, 8192]
    denom_i = sum_{j != i} exp(sim_ij / T)
    pos_i   = sim[i, i +- B]
    loss    = mean(log(denom_i) - pos_i / T)

Design (per core, identical SPMD program):
  - Host ships bf16 x in a column-tiled layout [n][p][c][col], ROTATED
    per core so the core's own 1024 rows are local column tiles 0-1
    (row sums are column-order invariant).
  - Norms WITHOUT the scalar engine: sq = x*x (DVE, all-bf16 fast
    path), ssq via ones-matmuls, then rsqrt by a Quake-style Newton
    iteration on DVE, done on a [128, W]-shaped copy of ssq obtained
    via a DRAM round-trip (the [1, N] shape would serialize on one
    DVE lane). rb = 128/||x||.
  - za8 = e4m3(x * rb) = e4m3(128 z): DVE multiply against the
    matmul-broadcast rb, fp8 output.
  - Main: 6 column groups (5x3 + 1x1 tiles) x 8 row tiles: fp8
    DoubleRow matmuls (K=256) into a [128, 3*512] PSUM group, one exp
    activation per group with accum_out row sums.
    exp(psum * 2^-13) = exp(sim/T).
  - The scalar engine runs ONLY the main exps and one final Ln: two
    activation-table loads total.
  - Epilogue: denom = rowsum - e^2 (self-term is constant 1 up to
    quantization, error ~1e-5 of the ~8200 denominator); one Ln.
    Positives need no log: sum_i pos_i is a single DVE multiply +
    reduce over za8 (mine tiles 0-1 x partner tiles 8-9).
  - Host: loss = (sum ln denom - (sum pospart)*2^-14/T) / 8192.
"""

import numpy as np
import ml_dtypes

B = 4096
D = 512
N = 2 * B            # 8192 rows of sim
NCORES = 8
RPC = N // NCORES    # 1024 rows per core
P = 128              # partitions
NT = 512             # column-tile width
NTILES = N // NT     # 16 column tiles
CT = D // P          # 4 contraction chunks of 128
ITILES = RPC // P    # 8 row tiles per core
GROUPS = [3, 3, 3, 3, 3, 1]          # n-tiles per psum group (sum 16)
NGRP = len(GROUPS)
# ssq scaled by 2^-14 before rsqrt -> rb = 128/||x||, za8 = e4m3(128 z)
SSQ_SCALE = 2.0 ** -14
# psum = za8 . za8 = 2^14 z.z ; exp(psum * EXP_SCALE) = exp(sim / T), T=0.5
EXP_SCALE = 2.0 ** -13
POS_SCALE = 2.0 ** -14
SELF_TERM = float(np.exp(2.0))   # exp(sim_ii / T), sim_ii = 1
INV_T = 2.0
MAGIC1 = 0x5F3759E0              # rsqrt magic + 1 (for ~x + magic + 1)

_CACHE = {}


def _build_bass():
    """Trace the per-core Bass program (identical for all 8 cores)."""
    import concourse.bass as bass
    import concourse.tile as tile
    from concourse import bacc, mybir

    dt = mybir.dt
    AF = mybir.ActivationFunctionType
    ALU = mybir.AluOpType
    DR = mybir.MatmulPerfMode.DoubleRow

    nc = bacc.Bacc(None, target_bir_lowering=False, debug=False, num_swdge_queues=4)

    xa_d = nc.dram_tensor("xa", [NTILES, P, CT, NT], dt.bfloat16,
                          kind="ExternalInput")
    out_d = nc.dram_tensor("loss_rows", [P, ITILES], dt.float32,
                           kind="ExternalOutput")
    pp_d = nc.dram_tensor("pospart", [P, 1], dt.float32, kind="ExternalOutput")
    dbg_d = nc.dram_tensor("dbg", [P, ITILES, 2], dt.float32,
                           kind="ExternalOutput")
    # DRAM bounce buffers for the [1, 2048] <-> [16, 128] reshape
    # (16 rows keeps DMA descriptors big; 128 partitions would mean 128
    # tiny descriptors and ~10us per bounce)
    batches = [(0, 4), (4, 8), (8, 12), (12, NTILES)]
    BP = 16                                   # bounce partitions
    scr_f = [nc.dram_tensor(f"scrf_{i}", [BP, (b1 - b0) * NT // BP], dt.float32,
                            kind="Internal") for i, (b0, b1) in enumerate(batches)]
    scr_b = [nc.dram_tensor(f"scrb_{i}", [BP, (b1 - b0) * NT // BP], dt.bfloat16,
                            kind="Internal") for i, (b0, b1) in enumerate(batches)]

    from contextlib import ExitStack

    with tile.TileContext(nc) as tc, ExitStack() as ctx:
        const = ctx.enter_context(tc.tile_pool(name="const", bufs=1))
        persist = ctx.enter_context(tc.tile_pool(name="persist", bufs=1))
        sqring = ctx.enter_context(tc.tile_pool(name="sqring", bufs=3))
        small = ctx.enter_context(tc.tile_pool(name="small", bufs=2))
        nwt = ctx.enter_context(tc.tile_pool(name="nwt", bufs=1))
        psum_pre = ctx.enter_context(tc.tile_pool(name="psum_pre", bufs=2,
                                                  space="PSUM"))
        psum_m = ctx.enter_context(tc.tile_pool(name="psum_m", bufs=2,
                                                space="PSUM"))

        ones_col = const.tile([P, 1], dt.bfloat16)
        nc.vector.memset(ones_col, 1.0)
        ones_row = const.tile([1, P], dt.bfloat16)
        nc.vector.memset(ones_row, 1.0)

        xa = persist.tile([P, NTILES, CT, NT], dt.bfloat16)
        za8 = persist.tile([P, NTILES, CT, NT], dt.float8e4)
        ssq_sb = persist.tile([1, N], dt.float32)
        rb_sb = persist.tile([1, N], dt.bfloat16)
        rowpart = persist.tile([P, ITILES, NGRP], dt.float32)

        # ---------------- input DMAs (all 16 tiles, pipelined) -----------
        # HWDGE via the idle sync engine: cheap triggers, and keeps the
        # gpsimd SWDGE queues free for the low-latency bounce DMAs.
        for n in range(NTILES):
            nc.sync.dma_start(out=xa[:, n, :, :], in_=xa_d[n])

        # ---------------- prologue helpers ------------------------------
        def ssq_tiles(tiles):
            """sq + ssq matmuls + scaled copy to ssq_sb for given tiles."""
            for n in tiles:
                sq = sqring.tile([P, CT, NT], dt.bfloat16,
                                 name=f"sq_{n}", tag="sq", bufs=3)
                nc.vector.tensor_mul(sq, xa[:, n, :, :], xa[:, n, :, :])
                ps = psum_pre.tile([P, NT], dt.float32,
                                   name=f"ssq_{n}", tag="pre")
                for c in range(CT):
                    nc.tensor.matmul(ps[0:1, :], ones_col, sq[:, c, :],
                                     start=(c == 0), stop=(c == CT - 1))
                nc.vector.tensor_scalar_mul(
                    ssq_sb[0:1, n * NT:(n + 1) * NT], ps[0:1, :], SSQ_SCALE)

        nwt_yt = {}

        def newton_fwd(bi):
            """Launch the ssq transpose bounce (latency hides under other
            DVE work emitted between fwd and compute)."""
            b0, b1 = batches[bi]
            lo, hi = b0 * NT, b1 * NT
            W = (hi - lo) // BP
            nc.gpsimd.dma_start(out=scr_f[bi][:], in_=ssq_sb[0:1, lo:hi])
            yt = nwt.tile([BP, W], dt.float32, name=f"yt_{bi}")
            nc.gpsimd.dma_start(out=yt, in_=scr_f[bi][:])
            nwt_yt[bi] = yt

        def newton_compute(bi):
            """rb = rsqrt(ssq*2^-14) on the [16, W] reshape."""
            b0, b1 = batches[bi]
            lo, hi = b0 * NT, b1 * NT
            W = (hi - lo) // BP
            yt = nwt_yt[bi]
            # y0 bits = MAGIC - (bits(y) >> 1), via float-domain value math
            # (DVE add is fp32 internally; the mod-2^32 trick would overflow)
            tu = nwt.tile([BP, W], dt.uint32, name=f"tu_{bi}")
            nc.vector.tensor_scalar(tu, yt.bitcast(dt.uint32), 1, None,
                                    op0=ALU.arith_shift_right)
            tf = nwt.tile([BP, W], dt.float32, name=f"tf_{bi}")
            nc.vector.tensor_copy(tf, tu)
            nc.vector.tensor_scalar(tf, tf, -1.0, float(MAGIC1 - 1),
                                    op0=ALU.mult, op1=ALU.add)
            y0u = nwt.tile([BP, W], dt.uint32, name=f"y0u_{bi}")
            nc.vector.tensor_copy(y0u, tf)
            h = nwt.tile([BP, W], dt.float32, name=f"h_{bi}")
            nc.vector.tensor_scalar_mul(h, yt, 0.5)
            y = y0u.bitcast(dt.float32)
            for it in range(2):
                t2 = nwt.tile([BP, W], dt.float32, name=f"t2_{bi}_{it}")
                nc.vector.tensor_mul(t2, y, y)
                nc.vector.tensor_mul(t2, t2, h)
                nc.vector.tensor_scalar(t2, t2, -1.0, 1.5,
                                        op0=ALU.mult, op1=ALU.add)
                yn = nwt.tile([BP, W], dt.float32, name=f"yn_{bi}_{it}")
                nc.vector.tensor_mul(yn, y, t2)
                y = yn
            rbT = nwt.tile([BP, W], dt.bfloat16, name=f"rbT_{bi}")
            nc.vector.tensor_copy(rbT, y)
            nc.gpsimd.dma_start(out=scr_b[bi][:], in_=rbT)
            nc.gpsimd.dma_start(out=rb_sb[0:1, lo:hi], in_=scr_b[bi][:])

        def za8_tiles(tiles):
            """broadcast rb + quantize za8 for given tiles."""
            for n in tiles:
                rbb = psum_pre.tile([P, NT], dt.float32,
                                    name=f"rbb_{n}", tag="pre")
                nc.tensor.matmul(rbb, ones_row,
                                 rb_sb[0:1, n * NT:(n + 1) * NT])
                rb_bc = bass.AP(tensor=rbb.tensor, offset=rbb.offset,
                                ap=[rbb.ap[0], [0, CT], rbb.ap[1]])
                nc.vector.tensor_mul(za8[:, n, :, :], xa[:, n, :, :], rb_bc)

        def main_group(g, n0, gsz):
            for i in range(ITILES):
                pm = psum_m.tile([P, 3, NT], dt.float32,
                                 name=f"pm_{g}_{i}", tag="pm")
                nm, mc = divmod(i, 4)
                for gg in range(2):
                    lhsT = za8[:, nm, 2 * gg:2 * gg + 2, mc * P:(mc + 1) * P]
                    for nn in range(gsz):
                        nc.tensor.matmul(pm[:, nn, :], lhsT,
                                         za8[:, n0 + nn, 2 * gg:2 * gg + 2, :],
                                         start=(gg == 0), stop=(gg == 1),
                                         perf_mode=DR)
                ej = sqring.tile([P, 3, NT], dt.bfloat16,
                                 name=f"ej_{g}_{i}", tag="ej", bufs=2)
                nc.scalar.activation(ej[:, 0:gsz, :], pm[:, 0:gsz, :], AF.Exp,
                                     scale=EXP_SCALE,
                                     accum_out=rowpart[:, i, g:g + 1])

        # ---------------- phase order ------------------------------------
        # All sq/ssq upfront (DMA-paced). Batch A's rb comes from the
        # scalar engine (idle before the main exps; avoids two ~5us DMA
        # bounce round-trips on the critical path). Batches B-D use the
        # DVE Newton whose bounce DMAs fly hidden under za8 work.
        gstart = np.cumsum([0] + GROUPS).tolist()

        ssq_tiles(range(*batches[0]))
        # batch A: rb = exp(-0.5 ln(ssq)) on the scalar engine (idle
        # before the main exps; avoids two DMA bounce round-trips on the
        # critical path)
        lnA = nwt.tile([1, batches[0][1] * NT], dt.float32, name="lnA")
        nc.scalar.activation(lnA, ssq_sb[0:1, 0:batches[0][1] * NT], AF.Ln)
        nc.scalar.activation(rb_sb[0:1, 0:batches[0][1] * NT], lnA, AF.Exp,
                             scale=-0.5)
        ssq_tiles(range(*batches[1]))
        newton_fwd(1)
        za8_tiles(range(*batches[0]))
        main_group(0, gstart[0], GROUPS[0])     # tiles 0-2
        newton_compute(1)
        za8_tiles(range(*batches[1]))
        ssq_tiles(range(*batches[2]))
        newton_fwd(2)
        main_group(1, gstart[1], GROUPS[1])     # tiles 3-5
        newton_compute(2)
        za8_tiles(range(*batches[2]))
        ssq_tiles(range(*batches[3]))
        newton_fwd(3)
        main_group(2, gstart[2], GROUPS[2])     # tiles 6-8
        newton_compute(3)
        za8_tiles(range(*batches[3]))
        for g in range(3, NGRP):
            main_group(g, gstart[g], GROUPS[g])

        # ---------------- positives partial (no log needed) --------------
        junk = small.tile([P, 2, CT, NT], dt.bfloat16)
        nc.vector.tensor_mul(junk, za8[:, 0:2, :, :], za8[:, 8:10, :, :])
        jf = bass.AP(tensor=junk.tensor, offset=junk.offset,
                     ap=[junk.ap[0], [1, 2 * CT * NT]])
        pp = small.tile([P, 1], dt.float32)
        nc.vector.tensor_reduce(pp, jf, axis=mybir.AxisListType.X, op=ALU.add)
        nc.gpsimd.dma_start(out=pp_d[:], in_=pp)

        # ---------------- epilogue: per-row log denominators -------------
        rowsum = small.tile([P, ITILES], dt.float32)
        nc.vector.tensor_reduce(rowsum, rowpart, axis=mybir.AxisListType.X,
                                op=ALU.add)
        denom = small.tile([P, ITILES], dt.float32)
        nc.vector.tensor_scalar_add(denom, rowsum, -SELF_TERM)
        loss_t = small.tile([P, ITILES], dt.float32)
        nc.scalar.activation(loss_t, denom, AF.Ln)
        nc.gpsimd.dma_start(out=out_d[:], in_=loss_t)

        dbg = small.tile([P, ITILES, 2], dt.float32)
        nc.vector.tensor_copy(dbg[:, :, 0], rowsum)
        nc.vector.tensor_copy(dbg[:, :, 1], denom)
        nc.gpsimd.dma_start(out=dbg_d[:], in_=dbg)

    nc.compile()
    return nc


def _get_nc():
    if "nc" not in _CACHE:
        _CACHE["nc"] = _build_bass()
    return _CACHE["nc"]


def _prep_inputs(polyline_embs, c_embs):
    """Host-side shard/tile prep. Returns in_maps for the 8 cores."""
    bf16 = ml_dtypes.bfloat16
    x = np.concatenate([np.asarray(polyline_embs, np.float32),
                        np.asarray(c_embs, np.float32)], axis=0)  # [8192, 512]
    xb = x.astype(bf16)

    in_maps = []
    for k in range(NCORES):
        qk = np.roll(xb, -k * RPC, axis=0)                        # mine first
        xt = np.ascontiguousarray(qk.T)                           # [512, 8192]
        xa = np.ascontiguousarray(
            xt.reshape(CT, P, NTILES, NT).transpose(2, 1, 0, 3))  # [16,128,4,512]
        in_maps.append({"xa": xa})
    return in_maps


def kernel(polyline_embs, c_embs):
    from concourse.bass_utils import run_bass_kernel_spmd

    nc = _get_nc()
    in_maps = _prep_inputs(polyline_embs, c_embs)
    res = run_bass_kernel_spmd(nc, in_maps, core_ids=list(range(NCORES)))
    _CACHE["last_results"] = res
    ln_total = 0.0
    pos_total = 0.0
    for r in res.results:
        ln_total += r["loss_rows"].astype(np.float64).sum()
        pos_total += r["pospart"].astype(np.float64).sum()
    loss = (ln_total - pos_total * POS_SCALE * INV_T) / N
    return np.float32(loss)


# revision 22
# speedup vs baseline: 1.0358x; 1.0358x over previous
"""InfoNCE loss kernel for Trainium2, 8 NeuronCores (v3).

Reference computation:
    z = l2_normalize(concat([polyline_embs, c_embs]))   # [8192, 512]
    sim = z @ z.T                                       # [8192, 8192]
    denom_i = sum_{j != i} exp(sim_ij / T)
    pos_i   = sim[i, i +- B]
    loss    = mean(log(denom_i) - pos_i / T)

Design (per core, identical SPMD program):
  - Host ships bf16 x in a column-tiled layout [n][p][c][col], ROTATED
    per core so the core's own 1024 rows are local column tiles 0-1
    (row sums are column-order invariant).
  - Norms WITHOUT the scalar engine: sq = x*x (DVE, all-bf16 fast
    path), ssq via ones-matmuls, then rsqrt by a Quake-style Newton
    iteration on DVE, done on a [128, W]-shaped copy of ssq obtained
    via a DRAM round-trip (the [1, N] shape would serialize on one
    DVE lane). rb = 128/||x||.
  - za8 = e4m3(x * rb) = e4m3(128 z): DVE multiply against the
    matmul-broadcast rb, fp8 output.
  - Main: 6 column groups (5x3 + 1x1 tiles) x 8 row tiles: fp8
    DoubleRow matmuls (K=256) into a [128, 3*512] PSUM group, one exp
    activation per group with accum_out row sums.
    exp(psum * 2^-13) = exp(sim/T).
  - The scalar engine runs ONLY the main exps and one final Ln: two
    activation-table loads total.
  - Epilogue: denom = rowsum - e^2 (self-term is constant 1 up to
    quantization, error ~1e-5 of the ~8200 denominator); one Ln.
    Positives need no log: sum_i pos_i is a single DVE multiply +
    reduce over za8 (mine tiles 0-1 x partner tiles 8-9).
  - Host: loss = (sum ln denom - (sum pospart)*2^-14/T) / 8192.
"""

import numpy as np
import ml_dtypes

B = 4096
D = 512
N = 2 * B            # 8192 rows of sim
NCORES = 8
RPC = N // NCORES    # 1024 rows per core
P = 128              # partitions
NT = 512             # column-tile width
NTILES = N // NT     # 16 column tiles
CT = D // P          # 4 contraction chunks of 128
ITILES = RPC // P    # 8 row tiles per core
GROUPS = [3, 3, 3, 3, 3, 1]          # n-tiles per psum group (sum 16)
NGRP = len(GROUPS)
# ssq scaled by 2^-14 before rsqrt -> rb = 128/||x||, za8 = e4m3(128 z)
SSQ_SCALE = 2.0 ** -14
# psum = za8 . za8 = 2^14 z.z ; exp(psum * EXP_SCALE) = exp(sim / T), T=0.5
EXP_SCALE = 2.0 ** -13
POS_SCALE = 2.0 ** -14
SELF_TERM = float(np.exp(2.0))   # exp(sim_ii / T), sim_ii = 1
INV_T = 2.0
MAGIC1 = 0x5F3759E0              # rsqrt magic + 1 (for ~x + magic + 1)

_CACHE = {}


def _build_bass():
    """Trace the per-core Bass program (identical for all 8 cores)."""
    import concourse.bass as bass
    import concourse.tile as tile
    from concourse import bacc, mybir

    dt = mybir.dt
    AF = mybir.ActivationFunctionType
    ALU = mybir.AluOpType
    DR = mybir.MatmulPerfMode.DoubleRow

    nc = bacc.Bacc(None, target_bir_lowering=False, debug=False, num_swdge_queues=4)

    xa_d = nc.dram_tensor("xa", [NTILES, P, CT, NT], dt.bfloat16,
                          kind="ExternalInput")
    out_d = nc.dram_tensor("loss_rows", [P, ITILES], dt.float32,
                           kind="ExternalOutput")
    pp_d = nc.dram_tensor("pospart", [P, 1], dt.float32, kind="ExternalOutput")
    dbg_d = nc.dram_tensor("dbg", [P, ITILES, 2], dt.float32,
                           kind="ExternalOutput")
    # DRAM bounce buffers for the [1, 2048] <-> [16, 128] reshape
    # (16 rows keeps DMA descriptors big; 128 partitions would mean 128
    # tiny descriptors and ~10us per bounce)
    batches = [(0, 4), (4, 8), (8, 12), (12, NTILES)]
    BP = 16                                   # bounce partitions
    scr_f = [nc.dram_tensor(f"scrf_{i}", [BP, (b1 - b0) * NT // BP], dt.float32,
                            kind="Internal") for i, (b0, b1) in enumerate(batches)]
    scr_b = [nc.dram_tensor(f"scrb_{i}", [BP, (b1 - b0) * NT // BP], dt.bfloat16,
                            kind="Internal") for i, (b0, b1) in enumerate(batches)]

    from contextlib import ExitStack

    with tile.TileContext(nc) as tc, ExitStack() as ctx:
        const = ctx.enter_context(tc.tile_pool(name="const", bufs=1))
        persist = ctx.enter_context(tc.tile_pool(name="persist", bufs=1))
        sqring = ctx.enter_context(tc.tile_pool(name="sqring", bufs=3))
        small = ctx.enter_context(tc.tile_pool(name="small", bufs=2))
        nwt = ctx.enter_context(tc.tile_pool(name="nwt", bufs=1))
        psum_pre = ctx.enter_context(tc.tile_pool(name="psum_pre", bufs=2,
                                                  space="PSUM"))
        psum_m = ctx.enter_context(tc.tile_pool(name="psum_m", bufs=2,
                                                space="PSUM"))

        ones_col = const.tile([P, 1], dt.bfloat16)
        nc.vector.memset(ones_col, 1.0)
        ones_row = const.tile([1, P], dt.bfloat16)
        nc.vector.memset(ones_row, 1.0)

        xa = persist.tile([P, NTILES, CT, NT], dt.bfloat16)
        za8 = persist.tile([P, NTILES, CT, NT], dt.float8e4)
        ssq_sb = persist.tile([1, N], dt.float32)
        rb_sb = persist.tile([1, N], dt.bfloat16)
        rowpart = persist.tile([P, ITILES, NGRP], dt.float32)

        # ---------------- input DMAs (all 16 tiles, pipelined) -----------
        # HWDGE via the idle sync engine: cheap triggers, and keeps the
        # gpsimd SWDGE queues free for the low-latency bounce DMAs.
        for n in range(NTILES):
            nc.sync.dma_start(out=xa[:, n, :, :], in_=xa_d[n])

        # ---------------- prologue helpers ------------------------------
        def ssq_tiles(tiles):
            """sq + ssq matmuls + scaled copy to ssq_sb for given tiles."""
            for n in tiles:
                sq = sqring.tile([P, CT, NT], dt.bfloat16,
                                 name=f"sq_{n}", tag="sq", bufs=3)
                nc.vector.tensor_mul(sq, xa[:, n, :, :], xa[:, n, :, :])
                ps = psum_pre.tile([P, NT], dt.float32,
                                   name=f"ssq_{n}", tag="pre")
                for c in range(CT):
                    nc.tensor.matmul(ps[0:1, :], ones_col, sq[:, c, :],
                                     start=(c == 0), stop=(c == CT - 1))
                nc.vector.tensor_scalar_mul(
                    ssq_sb[0:1, n * NT:(n + 1) * NT], ps[0:1, :], SSQ_SCALE)

        nwt_yt = {}

        def newton_fwd(bi):
            """Launch the ssq transpose bounce (latency hides under other
            DVE work emitted between fwd and compute)."""
            b0, b1 = batches[bi]
            lo, hi = b0 * NT, b1 * NT
            W = (hi - lo) // BP
            nc.gpsimd.dma_start(out=scr_f[bi][:], in_=ssq_sb[0:1, lo:hi])
            yt = nwt.tile([BP, W], dt.float32, name=f"yt_{bi}")
            nc.gpsimd.dma_start(out=yt, in_=scr_f[bi][:])
            nwt_yt[bi] = yt

        def newton_compute(bi):
            """rb = rsqrt(ssq*2^-14) on the [16, W] reshape."""
            b0, b1 = batches[bi]
            lo, hi = b0 * NT, b1 * NT
            W = (hi - lo) // BP
            yt = nwt_yt[bi]
            # y0 bits = MAGIC - (bits(y) >> 1), via float-domain value math
            # (DVE add is fp32 internally; the mod-2^32 trick would overflow)
            tu = nwt.tile([BP, W], dt.uint32, name=f"tu_{bi}")
            nc.vector.tensor_scalar(tu, yt.bitcast(dt.uint32), 1, None,
                                    op0=ALU.arith_shift_right)
            tf = nwt.tile([BP, W], dt.float32, name=f"tf_{bi}")
            nc.vector.tensor_copy(tf, tu)
            nc.vector.tensor_scalar(tf, tf, -1.0, float(MAGIC1 - 1),
                                    op0=ALU.mult, op1=ALU.add)
            y0u = nwt.tile([BP, W], dt.uint32, name=f"y0u_{bi}")
            nc.vector.tensor_copy(y0u, tf)
            h = nwt.tile([BP, W], dt.float32, name=f"h_{bi}")
            nc.vector.tensor_scalar_mul(h, yt, 0.5)
            y = y0u.bitcast(dt.float32)
            for it in range(2):
                t2 = nwt.tile([BP, W], dt.float32, name=f"t2_{bi}_{it}")
                nc.vector.tensor_mul(t2, y, y)
                nc.vector.tensor_mul(t2, t2, h)
                nc.vector.tensor_scalar(t2, t2, -1.0, 1.5,
                                        op0=ALU.mult, op1=ALU.add)
                yn = nwt.tile([BP, W], dt.float32, name=f"yn_{bi}_{it}")
                nc.vector.tensor_mul(yn, y, t2)
                y = yn
            rbT = nwt.tile([BP, W], dt.bfloat16, name=f"rbT_{bi}")
            nc.vector.tensor_copy(rbT, y)
            nc.gpsimd.dma_start(out=scr_b[bi][:], in_=rbT)
            nc.gpsimd.dma_start(out=rb_sb[0:1, lo:hi], in_=scr_b[bi][:])

        def za8_tiles(tiles):
            """broadcast rb + quantize za8 for given tiles."""
            for n in tiles:
                rbb = psum_pre.tile([P, NT], dt.float32,
                                    name=f"rbb_{n}", tag="pre")
                nc.tensor.matmul(rbb, ones_row,
                                 rb_sb[0:1, n * NT:(n + 1) * NT])
                rb_bc = bass.AP(tensor=rbb.tensor, offset=rbb.offset,
                                ap=[rbb.ap[0], [0, CT], rbb.ap[1]])
                nc.vector.tensor_mul(za8[:, n, :, :], xa[:, n, :, :], rb_bc)

        def main_group(g, n0, gsz):
            for i in range(ITILES):
                pm = psum_m.tile([P, 3, NT], dt.float32,
                                 name=f"pm_{g}_{i}", tag="pm")
                nm, mc = divmod(i, 4)
                for gg in range(2):
                    lhsT = za8[:, nm, 2 * gg:2 * gg + 2, mc * P:(mc + 1) * P]
                    for nn in range(gsz):
                        nc.tensor.matmul(pm[:, nn, :], lhsT,
                                         za8[:, n0 + nn, 2 * gg:2 * gg + 2, :],
                                         start=(gg == 0), stop=(gg == 1),
                                         perf_mode=DR)
                ej = sqring.tile([P, 3, NT], dt.bfloat16,
                                 name=f"ej_{g}_{i}", tag="ej", bufs=2)
                nc.scalar.activation(ej[:, 0:gsz, :], pm[:, 0:gsz, :], AF.Exp,
                                     scale=EXP_SCALE,
                                     accum_out=rowpart[:, i, g:g + 1])

        # ---------------- phase order ------------------------------------
        # All sq/ssq upfront (DMA-paced). Batch A's rb comes from the
        # scalar engine (idle before the main exps; avoids two ~5us DMA
        # bounce round-trips on the critical path). Batches B-D use the
        # DVE Newton whose bounce DMAs fly hidden under za8 work.
        gstart = np.cumsum([0] + GROUPS).tolist()

        ssq_tiles(range(*batches[0]))
        # batch A: rb = exp(-0.5 ln(ssq)) on the scalar engine (idle
        # before the main exps; avoids two DMA bounce round-trips on the
        # critical path)
        lnA = nwt.tile([1, batches[0][1] * NT], dt.float32, name="lnA")
        nc.scalar.activation(lnA, ssq_sb[0:1, 0:batches[0][1] * NT], AF.Ln)
        nc.scalar.activation(rb_sb[0:1, 0:batches[0][1] * NT], lnA, AF.Exp,
                             scale=-0.5)
        ssq_tiles(range(*batches[1]))
        newton_fwd(1)
        za8_tiles(range(*batches[0]))
        main_group(0, gstart[0], GROUPS[0])     # tiles 0-2
        newton_compute(1)
        za8_tiles(range(*batches[1]))
        main_group(1, gstart[1], GROUPS[1])     # tiles 3-5
        ssq_tiles(range(*batches[2]))
        newton_fwd(2)
        ssq_tiles(range(*batches[3]))
        newton_fwd(3)
        newton_compute(2)
        za8_tiles(range(*batches[2]))
        main_group(2, gstart[2], GROUPS[2])     # tiles 6-8
        newton_compute(3)
        za8_tiles(range(*batches[3]))
        for g in range(3, NGRP):
            main_group(g, gstart[g], GROUPS[g])

        # ---------------- positives partial (no log needed) --------------
        junk = small.tile([P, 2, CT, NT], dt.bfloat16)
        nc.vector.tensor_mul(junk, za8[:, 0:2, :, :], za8[:, 8:10, :, :])
        jf = bass.AP(tensor=junk.tensor, offset=junk.offset,
                     ap=[junk.ap[0], [1, 2 * CT * NT]])
        pp = small.tile([P, 1], dt.float32)
        nc.vector.tensor_reduce(pp, jf, axis=mybir.AxisListType.X, op=ALU.add)
        nc.gpsimd.dma_start(out=pp_d[:], in_=pp)

        # ---------------- epilogue: per-row log denominators -------------
        rowsum = small.tile([P, ITILES], dt.float32)
        nc.vector.tensor_reduce(rowsum, rowpart, axis=mybir.AxisListType.X,
                                op=ALU.add)
        denom = small.tile([P, ITILES], dt.float32)
        nc.vector.tensor_scalar_add(denom, rowsum, -SELF_TERM)
        loss_t = small.tile([P, ITILES], dt.float32)
        nc.scalar.activation(loss_t, denom, AF.Ln)
        nc.gpsimd.dma_start(out=out_d[:], in_=loss_t)

        dbg = small.tile([P, ITILES, 2], dt.float32)
        nc.vector.tensor_copy(dbg[:, :, 0], rowsum)
        nc.vector.tensor_copy(dbg[:, :, 1], denom)
        nc.gpsimd.dma_start(out=dbg_d[:], in_=dbg)

    nc.compile()
    return nc


def _get_nc():
    if "nc" not in _CACHE:
        _CACHE["nc"] = _build_bass()
    return _CACHE["nc"]


def _prep_inputs(polyline_embs, c_embs):
    """Host-side shard/tile prep. Returns in_maps for the 8 cores."""
    bf16 = ml_dtypes.bfloat16
    x = np.concatenate([np.asarray(polyline_embs, np.float32),
                        np.asarray(c_embs, np.float32)], axis=0)  # [8192, 512]
    xb = x.astype(bf16)

    in_maps = []
    for k in range(NCORES):
        qk = np.roll(xb, -k * RPC, axis=0)                        # mine first
        xt = np.ascontiguousarray(qk.T)                           # [512, 8192]
        xa = np.ascontiguousarray(
            xt.reshape(CT, P, NTILES, NT).transpose(2, 1, 0, 3))  # [16,128,4,512]
        in_maps.append({"xa": xa})
    return in_maps


def kernel(polyline_embs, c_embs):
    from concourse.bass_utils import run_bass_kernel_spmd

    nc = _get_nc()
    in_maps = _prep_inputs(polyline_embs, c_embs)
    res = run_bass_kernel_spmd(nc, in_maps, core_ids=list(range(NCORES)))
    _CACHE["last_results"] = res
    ln_total = 0.0
    pos_total = 0.0
    for r in res.results:
        ln_total += r["loss_rows"].astype(np.float64).sum()
        pos_total += r["pospart"].astype(np.float64).sum()
    loss = (ln_total - pos_total * POS_SCALE * INV_T) / N
    return np.float32(loss)


# revision 23
# speedup vs baseline: 1.1067x; 1.0685x over previous
"""InfoNCE loss kernel for Trainium2, 8 NeuronCores (v3).

Reference computation:
    z = l2_normalize(concat([polyline_embs, c_embs]))   # [8192, 512]
    sim = z @ z.T                                       # [8192, 8192]
    denom_i = sum_{j != i} exp(sim_ij / T)
    pos_i   = sim[i, i +- B]
    loss    = mean(log(denom_i) - pos_i / T)

Design (per core, identical SPMD program):
  - Host ships bf16 x in a column-tiled layout [n][p][c][col], ROTATED
    per core so the core's own 1024 rows are local column tiles 0-1
    (row sums are column-order invariant).
  - Norms WITHOUT the scalar engine: sq = x*x (DVE, all-bf16 fast
    path), ssq via ones-matmuls, then rsqrt by a Quake-style Newton
    iteration on DVE, done on a [128, W]-shaped copy of ssq obtained
    via a DRAM round-trip (the [1, N] shape would serialize on one
    DVE lane). rb = 128/||x||.
  - za8 = e4m3(x * rb) = e4m3(128 z): DVE multiply against the
    matmul-broadcast rb, fp8 output.
  - Main: 6 column groups (5x3 + 1x1 tiles) x 8 row tiles: fp8
    DoubleRow matmuls (K=256) into a [128, 3*512] PSUM group, one exp
    activation per group with accum_out row sums.
    exp(psum * 2^-13) = exp(sim/T).
  - The scalar engine runs ONLY the main exps and one final Ln: two
    activation-table loads total.
  - Epilogue: denom = rowsum - e^2 (self-term is constant 1 up to
    quantization, error ~1e-5 of the ~8200 denominator); one Ln.
    Positives need no log: sum_i pos_i is a single DVE multiply +
    reduce over za8 (mine tiles 0-1 x partner tiles 8-9).
  - Host: loss = (sum ln denom - (sum pospart)*2^-14/T) / 8192.
"""

import numpy as np
import ml_dtypes

B = 4096
D = 512
N = 2 * B            # 8192 rows of sim
NCORES = 8
RPC = N // NCORES    # 1024 rows per core
P = 128              # partitions
NT = 512             # column-tile width
NTILES = N // NT     # 16 column tiles
CT = D // P          # 4 contraction chunks of 128
ITILES = RPC // P    # 8 row tiles per core
GROUPS = [3, 3, 3, 3, 3, 1]          # n-tiles per psum group (sum 16)
NGRP = len(GROUPS)
# ssq scaled by 2^-14 before rsqrt -> rb = 128/||x||, za8 = e4m3(128 z)
SSQ_SCALE = 2.0 ** -14
# psum = za8 . za8 = 2^14 z.z ; exp(psum * EXP_SCALE) = exp(sim / T), T=0.5
EXP_SCALE = 2.0 ** -13
POS_SCALE = 2.0 ** -14
SELF_TERM = float(np.exp(2.0))   # exp(sim_ii / T), sim_ii = 1
INV_T = 2.0
MAGIC1 = 0x5F3759E0              # rsqrt magic + 1 (for ~x + magic + 1)

_CACHE = {}


def _build_bass():
    """Trace the per-core Bass program (identical for all 8 cores)."""
    import concourse.bass as bass
    import concourse.tile as tile
    from concourse import bacc, mybir

    dt = mybir.dt
    AF = mybir.ActivationFunctionType
    ALU = mybir.AluOpType
    DR = mybir.MatmulPerfMode.DoubleRow

    nc = bacc.Bacc(None, target_bir_lowering=False, debug=False, num_swdge_queues=4)

    xa_d = nc.dram_tensor("xa", [NTILES, P, CT, NT], dt.bfloat16,
                          kind="ExternalInput")
    out_d = nc.dram_tensor("loss_rows", [P, ITILES], dt.float32,
                           kind="ExternalOutput")
    pp_d = nc.dram_tensor("pospart", [P, 1], dt.float32, kind="ExternalOutput")
    dbg_d = nc.dram_tensor("dbg", [P, ITILES, 2], dt.float32,
                           kind="ExternalOutput")
    # DRAM bounce buffers for the [1, 2048] <-> [16, 128] reshape
    # (16 rows keeps DMA descriptors big; 128 partitions would mean 128
    # tiny descriptors and ~10us per bounce)
    batches = [(0, 4), (4, 8), (8, 12), (12, NTILES)]
    BP = 16                                   # bounce partitions
    scr_f = [nc.dram_tensor(f"scrf_{i}", [BP, (b1 - b0) * NT // BP], dt.float32,
                            kind="Internal") for i, (b0, b1) in enumerate(batches)]
    scr_b = [nc.dram_tensor(f"scrb_{i}", [BP, (b1 - b0) * NT // BP], dt.bfloat16,
                            kind="Internal") for i, (b0, b1) in enumerate(batches)]

    from contextlib import ExitStack

    with tile.TileContext(nc) as tc, ExitStack() as ctx:
        const = ctx.enter_context(tc.tile_pool(name="const", bufs=1))
        persist = ctx.enter_context(tc.tile_pool(name="persist", bufs=1))
        sqring = ctx.enter_context(tc.tile_pool(name="sqring", bufs=3))
        small = ctx.enter_context(tc.tile_pool(name="small", bufs=2))
        nwt = ctx.enter_context(tc.tile_pool(name="nwt", bufs=1))
        psum_pre = ctx.enter_context(tc.tile_pool(name="psum_pre", bufs=2,
                                                  space="PSUM"))
        psum_m = ctx.enter_context(tc.tile_pool(name="psum_m", bufs=2,
                                                space="PSUM"))

        ones_col = const.tile([P, 1], dt.bfloat16)
        nc.vector.memset(ones_col, 1.0)
        ones_row = const.tile([1, P], dt.bfloat16)
        nc.vector.memset(ones_row, 1.0)

        xa = persist.tile([P, NTILES, CT, NT], dt.bfloat16)
        za8 = persist.tile([P, NTILES, CT, NT], dt.float8e4)
        ssq_sb = persist.tile([1, N], dt.float32)
        rb_sb = persist.tile([1, N], dt.bfloat16)
        rowpart = persist.tile([P, ITILES, NGRP], dt.float32)

        # ---------------- input DMAs (all 16 tiles, pipelined) -----------
        # HWDGE via the idle sync engine: cheap triggers, and keeps the
        # gpsimd SWDGE queues free for the low-latency bounce DMAs.
        for n in range(NTILES):
            nc.sync.dma_start(out=xa[:, n, :, :], in_=xa_d[n])

        # ---------------- prologue helpers ------------------------------
        def ssq_tiles(tiles):
            """sq + ssq matmuls + scaled copy to ssq_sb for given tiles."""
            for n in tiles:
                sq = sqring.tile([P, CT, NT], dt.bfloat16,
                                 name=f"sq_{n}", tag="sq", bufs=3)
                nc.vector.tensor_mul(sq, xa[:, n, :, :], xa[:, n, :, :])
                ps = psum_pre.tile([P, NT], dt.float32,
                                   name=f"ssq_{n}", tag="pre")
                for c in range(CT):
                    nc.tensor.matmul(ps[0:1, :], ones_col, sq[:, c, :],
                                     start=(c == 0), stop=(c == CT - 1))
                nc.vector.tensor_scalar_mul(
                    ssq_sb[0:1, n * NT:(n + 1) * NT], ps[0:1, :], SSQ_SCALE)

        nwt_yt = {}

        def newton_fwd(bi):
            """Launch the ssq transpose bounce (latency hides under other
            DVE work emitted between fwd and compute)."""
            b0, b1 = batches[bi]
            lo, hi = b0 * NT, b1 * NT
            W = (hi - lo) // BP
            nc.gpsimd.dma_start(out=scr_f[bi][:], in_=ssq_sb[0:1, lo:hi])
            yt = nwt.tile([BP, W], dt.float32, name=f"yt_{bi}")
            nc.gpsimd.dma_start(out=yt, in_=scr_f[bi][:])
            nwt_yt[bi] = yt

        def newton_compute(bi):
            """rb = rsqrt(ssq*2^-14) on the [16, W] reshape."""
            b0, b1 = batches[bi]
            lo, hi = b0 * NT, b1 * NT
            W = (hi - lo) // BP
            yt = nwt_yt[bi]
            # y0 bits = MAGIC - (bits(y) >> 1), via float-domain value math
            # (DVE add is fp32 internally; the mod-2^32 trick would overflow)
            tu = nwt.tile([BP, W], dt.uint32, name=f"tu_{bi}")
            nc.vector.tensor_scalar(tu, yt.bitcast(dt.uint32), 1, None,
                                    op0=ALU.arith_shift_right)
            tf = nwt.tile([BP, W], dt.float32, name=f"tf_{bi}")
            nc.vector.tensor_copy(tf, tu)
            nc.vector.tensor_scalar(tf, tf, -1.0, float(MAGIC1 - 1),
                                    op0=ALU.mult, op1=ALU.add)
            y0u = nwt.tile([BP, W], dt.uint32, name=f"y0u_{bi}")
            nc.vector.tensor_copy(y0u, tf)
            h = nwt.tile([BP, W], dt.float32, name=f"h_{bi}")
            nc.vector.tensor_scalar_mul(h, yt, 0.5)
            y = y0u.bitcast(dt.float32)
            for it in range(2):
                t2 = nwt.tile([BP, W], dt.float32, name=f"t2_{bi}_{it}")
                nc.vector.tensor_mul(t2, y, y)
                nc.vector.tensor_mul(t2, t2, h)
                nc.vector.tensor_scalar(t2, t2, -1.0, 1.5,
                                        op0=ALU.mult, op1=ALU.add)
                yn = nwt.tile([BP, W], dt.float32, name=f"yn_{bi}_{it}")
                nc.vector.tensor_mul(yn, y, t2)
                y = yn
            rbT = nwt.tile([BP, W], dt.bfloat16, name=f"rbT_{bi}")
            nc.vector.tensor_copy(rbT, y)
            nc.gpsimd.dma_start(out=scr_b[bi][:], in_=rbT)
            nc.gpsimd.dma_start(out=rb_sb[0:1, lo:hi], in_=scr_b[bi][:])

        def za8_tiles(tiles):
            """broadcast rb + quantize za8 for given tiles."""
            for n in tiles:
                rbb = psum_pre.tile([P, NT], dt.float32,
                                    name=f"rbb_{n}", tag="pre")
                nc.tensor.matmul(rbb, ones_row,
                                 rb_sb[0:1, n * NT:(n + 1) * NT])
                rb_bc = bass.AP(tensor=rbb.tensor, offset=rbb.offset,
                                ap=[rbb.ap[0], [0, CT], rbb.ap[1]])
                nc.vector.tensor_mul(za8[:, n, :, :], xa[:, n, :, :], rb_bc)

        def main_group(g, n0, gsz):
            for i in range(ITILES):
                pm = psum_m.tile([P, 3, NT], dt.float32,
                                 name=f"pm_{g}_{i}", tag="pm")
                nm, mc = divmod(i, 4)
                for gg in range(2):
                    lhsT = za8[:, nm, 2 * gg:2 * gg + 2, mc * P:(mc + 1) * P]
                    for nn in range(gsz):
                        nc.tensor.matmul(pm[:, nn, :], lhsT,
                                         za8[:, n0 + nn, 2 * gg:2 * gg + 2, :],
                                         start=(gg == 0), stop=(gg == 1),
                                         perf_mode=DR)
                ej = sqring.tile([P, 3, NT], dt.bfloat16,
                                 name=f"ej_{g}_{i}", tag="ej", bufs=2)
                nc.scalar.activation(ej[:, 0:gsz, :], pm[:, 0:gsz, :], AF.Exp,
                                     scale=EXP_SCALE,
                                     accum_out=rowpart[:, i, g:g + 1])

        # ---------------- phase order ------------------------------------
        # All sq/ssq upfront (DMA-paced). Batch A's rb comes from the
        # scalar engine (idle before the main exps; avoids two ~5us DMA
        # bounce round-trips on the critical path). Batches B-D use the
        # DVE Newton whose bounce DMAs fly hidden under za8 work.
        gstart = np.cumsum([0] + GROUPS).tolist()

        for bi in range(4):
            ssq_tiles(range(*batches[bi]))
            if bi > 0:
                newton_fwd(bi)

        # batch A: rb = exp(-0.5 ln(ssq)) on the scalar engine (idle
        # before the main exps; avoids two DMA bounce round-trips on the
        # critical path)
        lnA = nwt.tile([1, batches[0][1] * NT], dt.float32, name="lnA")
        nc.scalar.activation(lnA, ssq_sb[0:1, 0:batches[0][1] * NT], AF.Ln)
        nc.scalar.activation(rb_sb[0:1, 0:batches[0][1] * NT], lnA, AF.Exp,
                             scale=-0.5)
        za8_tiles(range(*batches[0]))
        main_group(0, gstart[0], GROUPS[0])     # tiles 0-2
        newton_compute(1)
        za8_tiles(range(*batches[1]))
        main_group(1, gstart[1], GROUPS[1])     # tiles 3-5
        newton_compute(2)
        za8_tiles(range(*batches[2]))
        main_group(2, gstart[2], GROUPS[2])     # tiles 6-8
        newton_compute(3)
        za8_tiles(range(*batches[3]))
        for g in range(3, NGRP):
            main_group(g, gstart[g], GROUPS[g])

        # ---------------- positives partial (no log needed) --------------
        junk = small.tile([P, 2, CT, NT], dt.bfloat16)
        nc.vector.tensor_mul(junk, za8[:, 0:2, :, :], za8[:, 8:10, :, :])
        jf = bass.AP(tensor=junk.tensor, offset=junk.offset,
                     ap=[junk.ap[0], [1, 2 * CT * NT]])
        pp = small.tile([P, 1], dt.float32)
        nc.vector.tensor_reduce(pp, jf, axis=mybir.AxisListType.X, op=ALU.add)
        nc.gpsimd.dma_start(out=pp_d[:], in_=pp)

        # ---------------- epilogue: per-row log denominators -------------
        rowsum = small.tile([P, ITILES], dt.float32)
        nc.vector.tensor_reduce(rowsum, rowpart, axis=mybir.AxisListType.X,
                                op=ALU.add)
        denom = small.tile([P, ITILES], dt.float32)
        nc.vector.tensor_scalar_add(denom, rowsum, -SELF_TERM)
        loss_t = small.tile([P, ITILES], dt.float32)
        nc.scalar.activation(loss_t, denom, AF.Ln)
        nc.gpsimd.dma_start(out=out_d[:], in_=loss_t)

        dbg = small.tile([P, ITILES, 2], dt.float32)
        nc.vector.tensor_copy(dbg[:, :, 0], rowsum)
        nc.vector.tensor_copy(dbg[:, :, 1], denom)
        nc.gpsimd.dma_start(out=dbg_d[:], in_=dbg)

    nc.compile()
    return nc


def _get_nc():
    if "nc" not in _CACHE:
        _CACHE["nc"] = _build_bass()
    return _CACHE["nc"]


def _prep_inputs(polyline_embs, c_embs):
    """Host-side shard/tile prep. Returns in_maps for the 8 cores."""
    bf16 = ml_dtypes.bfloat16
    x = np.concatenate([np.asarray(polyline_embs, np.float32),
                        np.asarray(c_embs, np.float32)], axis=0)  # [8192, 512]
    xb = x.astype(bf16)

    in_maps = []
    for k in range(NCORES):
        qk = np.roll(xb, -k * RPC, axis=0)                        # mine first
        xt = np.ascontiguousarray(qk.T)                           # [512, 8192]
        xa = np.ascontiguousarray(
            xt.reshape(CT, P, NTILES, NT).transpose(2, 1, 0, 3))  # [16,128,4,512]
        in_maps.append({"xa": xa})
    return in_maps


def kernel(polyline_embs, c_embs):
    from concourse.bass_utils import run_bass_kernel_spmd

    nc = _get_nc()
    in_maps = _prep_inputs(polyline_embs, c_embs)
    res = run_bass_kernel_spmd(nc, in_maps, core_ids=list(range(NCORES)))
    _CACHE["last_results"] = res
    ln_total = 0.0
    pos_total = 0.0
    for r in res.results:
        ln_total += r["loss_rows"].astype(np.float64).sum()
        pos_total += r["pospart"].astype(np.float64).sum()
    loss = (ln_total - pos_total * POS_SCALE * INV_T) / N
    return np.float32(loss)
